# revision 1
# baseline (speedup 1.0000x reference)
"""Trainium2 Bass kernel for nn_LocalAggregation (ball-query KNN + grouped MLP + max-pool).

Math refactor: with BN in eval mode and ReLU/max commuting past the per-query
affine part, the whole conv+BN+ReLU+max collapses to
    out[c, m] = relu( max_{k in NN32(m)} Gt[k, c]  -  Ht[m, c] )
where Gt = (diag(s)@W) @ [fea; xyz/R]  per point, Ht per query,
s = gamma/sqrt(var+eps), and the ball-query mask replaces far neighbors with
the nearest (self) index.

Sharding: 8 cores = 4 batches x 2 query-halves. Each core handles 4096
queries x all 8192 keys of one batch sample.
"""

import numpy as np

import concourse.bacc as bacc
import concourse.mybir as mybir
from concourse import tile
from concourse.bass_utils import run_bass_kernel_spmd

B, C, N = 4, 64, 8192
K = 32
RADIUS = 0.2
R2 = RADIUS * RADIUS
EPS = 1e-5
CIN = C + 3            # 67
NCORES = 8
QPC = N // 2           # queries per core
NT = QPC // 128        # query tiles per core (32)
NBLK = N // 128        # key blocks per tile (64)
NCAND = NBLK * 8       # candidates per query (512)
NEG = -3.0e38

f32 = mybir.dt.float32
u16 = mybir.dt.uint16
u32 = mybir.dt.uint32

_CACHE = {}


def _build(debug=False):
    nc = bacc.Bacc("TRN2", target_bir_lowering=False, debug=False,
                   num_devices=NCORES)

    xyz_in = nc.dram_tensor("xyz", [3, N], f32, kind="ExternalInput").ap()
    xyzq_in = nc.dram_tensor("xyzq", [3, QPC], f32, kind="ExternalInput").ap()
    fea_in = nc.dram_tensor("fea", [C, N], f32, kind="ExternalInput").ap()
    w_in = nc.dram_tensor("w", [C, CIN], f32, kind="ExternalInput").ap()
    bnt_in = nc.dram_tensor("bnt", [C, 4], f32, kind="ExternalInput").ap()
    blockbase_in = nc.dram_tensor("blockbase", [128, NCAND], u16,
                                  kind="ExternalInput").ap()
    ranks_in = nc.dram_tensor("ranks", [128, K], u16,
                              kind="ExternalInput").ap()
    constrows_in = nc.dram_tensor("constrows", [2, N], f32,
                                  kind="ExternalInput").ap()
    y_out = nc.dram_tensor("y", [QPC, C], f32, kind="ExternalOutput").ap()
    if debug:
        dbg = {
            "d_dist": nc.dram_tensor("d_dist", [128, N], f32, kind="ExternalOutput").ap(),
            "d_cand": nc.dram_tensor("d_cand", [128, NCAND], f32, kind="ExternalOutput").ap(),
            "d_gcand": nc.dram_tensor("d_gcand", [128, NCAND], u16, kind="ExternalOutput").ap(),
            "d_mxc": nc.dram_tensor("d_mxc", [128, K], f32, kind="ExternalOutput").ap(),
            "d_pos": nc.dram_tensor("d_pos", [128, K], u16, kind="ExternalOutput").ap(),
            "d_gidx": nc.dram_tensor("d_gidx", [128, K], u16, kind="ExternalOutput").ap(),
            "d_gath": nc.dram_tensor("d_gath", [128, K * C], f32, kind="ExternalOutput").ap(),
            "d_gt": nc.dram_tensor("d_gt", [N, C], f32, kind="ExternalOutput").ap(),
        }

    sq_dram = nc.dram_tensor("sq_scr", [1, N], f32).ap()
    nsq_dram = nc.dram_tensor("nsq_scr", [1, N], f32).ap()
    sqq_dram = nc.dram_tensor("sqq_scr", [1, QPC], f32).ap()
    cc_dram = nc.dram_tensor("cc_scr", [C, 1], f32).ap()
    wct_dram = nc.dram_tensor("wct_scr", [3, C], f32).ap()
    gt_dram = nc.dram_tensor("gt", [N, C], f32).ap()
    idx_dram = nc.dram_tensor("idxb", [NT * K * 128], u16).ap()

    with tile.TileContext(nc) as tc:
        # ---------------- persistent tiles ----------------
        with tc.tile_pool(name="persist", bufs=1) as pp:
            a5 = pp.tile([5, QPC], f32, tag="a5")        # query side lhsT rows
            a4 = pp.tile([4, QPC], f32, tag="a4")        # Ht lhsT rows
            b5 = pp.tile([5, N], f32, tag="b5")          # key side rhs rows
            rhs4 = pp.tile([4, C], f32, tag="rhs4")      # [Wc'/R ; cc] rhs
            blockbase = pp.tile([128, NCAND], u16, tag="bb")
            ranks = pp.tile([128, K], u16, tag="ranks")
            nc.sync.dma_start(out=blockbase[:], in_=blockbase_in[:])
            nc.sync.dma_start(out=ranks[:], in_=ranks_in[:])

            # ---------------- prep ----------------
            with tc.tile_pool(name="prep", bufs=1) as sp, \
                 tc.tile_pool(name="prep_ps", bufs=2, space="PSUM") as pps:
                f67 = sp.tile([CIN, N], f32)
                w = sp.tile([C, CIN], f32)
                bnt = sp.tile([C, 4], f32)
                nc.sync.dma_start(out=f67[:C, :], in_=fea_in[:])
                nc.sync.dma_start(out=f67[C:, :], in_=xyz_in[:])
                nc.sync.dma_start(out=w[:], in_=w_in[:])
                nc.sync.dma_start(out=bnt[:], in_=bnt_in[:])
                # load query coords early (a5 rows 0-2 double as pristine xyzq)
                nc.sync.dma_start(out=a5[0:3, :], in_=xyzq_in[:])
                nc.sync.dma_start(out=a4[0:3, :], in_=xyzq_in[:])

                # s = gamma / sqrt(var + eps); cc = s*mean - beta   (per channel)
                s_t = sp.tile([C, 1], f32)
                tmp = sp.tile([C, 1], f32)
                nc.vector.tensor_scalar_add(tmp[:], bnt[:, 3:4], EPS)
                nc.scalar.activation(tmp[:], tmp[:],
                                     mybir.ActivationFunctionType.Sqrt)
                nc.vector.reciprocal(tmp[:], tmp[:])
                nc.vector.tensor_mul(s_t[:], bnt[:, 0:1], tmp[:])
                cc_t = sp.tile([C, 1], f32)
                nc.vector.tensor_mul(cc_t[:], bnt[:, 2:3], s_t[:])
                nc.vector.tensor_sub(cc_t[:], cc_t[:], bnt[:, 1:2])
                nc.sync.dma_start(out=cc_dram[:], in_=cc_t[:])

                # W' = diag(s) @ W ; coor columns additionally * (1/R)
                wp = sp.tile([C, CIN], f32)
                nc.vector.tensor_scalar_mul(wp[:], w[:], s_t[:])
                nc.vector.tensor_scalar_mul(wp[:, C:], wp[:, C:], 1.0 / RADIUS)

                # diag(s') trick not needed: transpose W' via matmul with diag.
                # Instead compute W'T = lhsT(W').T @ I  using tensor.transpose
                # would need identity; cheaper: W'T[k, c] = sum_p W'[p, k] * D[p, c]
                diag = sp.tile([C, C], f32)
                nc.gpsimd.memset(diag[:], 0.0)
                one_col = sp.tile([C, 1], f32)
                nc.gpsimd.memset(one_col[:], 1.0)
                nc.gpsimd.affine_select(
                    diag[:], one_col[:].to_broadcast([C, C]),
                    pattern=[[-1, C]], base=0, channel_multiplier=1,
                    compare_op=mybir.AluOpType.is_equal, fill=0.0)
                wpt_ps = pps.tile([CIN, C], f32)
                nc.tensor.matmul(wpt_ps[:], wp[:], diag[:], start=True, stop=True)
                wpt = sp.tile([CIN, C], f32)
                nc.scalar.copy(wpt[:], wpt_ps[:])
                # stash coor rows of W'T for rhs4 assembly (partition shift via DRAM)
                nc.sync.dma_start(out=wct_dram[:], in_=wpt[C:, :])

                # sq = sum(xyz^2) along the 3 coords (PE ones-reduction), chunked
                ones3 = sp.tile([3, 1], f32)
                nc.gpsimd.memset(ones3[:], 1.0)
                for k in range(N // 512):
                    t3 = sp.tile([3, 512], f32, tag="t3")
                    nc.vector.tensor_mul(t3[:], f67[C:, k * 512:(k + 1) * 512],
                                         f67[C:, k * 512:(k + 1) * 512])
                    ps = pps.tile([1, 512], f32, tag="sqps")
                    nc.tensor.matmul(ps[:], ones3[:], t3[:], start=True, stop=True)
                    sqc = sp.tile([1, 512], f32, tag="sqc")
                    nc.scalar.copy(sqc[:], ps[:])
                    nc.sync.dma_start(out=sq_dram[:, k * 512:(k + 1) * 512],
                                      in_=sqc[:])
                    nsqc = sp.tile([1, 512], f32, tag="nsqc")
                    nc.vector.tensor_scalar_mul(nsqc[:], sqc[:], -1.0)
                    nc.sync.dma_start(out=nsq_dram[:, k * 512:(k + 1) * 512],
                                      in_=nsqc[:])
                for k in range(QPC // 512):
                    t3 = sp.tile([3, 512], f32, tag="t3")
                    nc.vector.tensor_mul(t3[:], a5[0:3, k * 512:(k + 1) * 512],
                                         a5[0:3, k * 512:(k + 1) * 512])
                    ps = pps.tile([1, 512], f32, tag="sqps")
                    nc.tensor.matmul(ps[:], ones3[:], t3[:], start=True, stop=True)
                    sqc = sp.tile([1, 512], f32, tag="sqc")
                    nc.scalar.copy(sqc[:], ps[:])
                    nc.sync.dma_start(out=sqq_dram[:, k * 512:(k + 1) * 512],
                                      in_=sqc[:])

                # B5 = [2x, 2y, 2z, -1, -sq] over keys
                nc.sync.dma_start(out=b5[0:3, :], in_=xyz_in[:])
                nc.vector.tensor_scalar_mul(b5[0:3, :], b5[0:3, :], 2.0)
                nc.sync.dma_start(out=b5[3:4, :], in_=constrows_in[1:2, :])
                nc.sync.dma_start(out=b5[4:5, :], in_=nsq_dram[:])

                # A5 rows: [x, y, z, sq, 1] over queries ; A4 row 3 = 1
                nc.sync.dma_start(out=a5[3:4, :], in_=sqq_dram[:])
                nc.sync.dma_start(out=a5[4:5, :], in_=constrows_in[0:1, :QPC])
                nc.sync.dma_start(out=a4[3:4, :], in_=constrows_in[0:1, :QPC])

                # rhs4 = [W'T coor rows ; ccT]
                nc.sync.dma_start(out=rhs4[0:3, :], in_=wct_dram[:])
                nc.sync.dma_start(out=rhs4[3:4, :],
                                  in_=cc_dram[:].rearrange("c one -> one c"))

                # Gt[n, c] = sum_p F67[p, n] * W'T[p, c]  -> DRAM [N, C]
                gstage = sp.tile([128, (N // 128) * C], f32)
                for blk in range(N // 128):
                    gps = pps.tile([128, C], f32, tag="gps")
                    nc.tensor.matmul(gps[:], f67[:, blk * 128:(blk + 1) * 128],
                                     wpt[:], start=True, stop=True)
                    nc.scalar.copy(gstage[:, blk * C:(blk + 1) * C], gps[:])
                nc.sync.dma_start(
                    out=gt_dram[:].rearrange("(blk p) c -> p blk c", p=128),
                    in_=gstage[:].rearrange("p (blk c) -> p blk c", c=C))

            # ---------------- phase A: selection over query tiles ----------------
            last_ls = None
            with tc.tile_pool(name="nd_ps", bufs=6, space="PSUM") as ndp, \
                 tc.tile_pool(name="dist", bufs=2) as dp, \
                 tc.tile_pool(name="small", bufs=2) as smp:
                for t in range(NT):
                    q0 = t * 128
                    dist = dp.tile([128, N], f32, tag="dist")
                    for k in range(N // 512):
                        ps = ndp.tile([128, 512], f32, tag="nd")
                        nc.tensor.matmul(ps[:], a5[:, q0:q0 + 128],
                                         b5[:, k * 512:(k + 1) * 512],
                                         start=True, stop=True)
                        nc.scalar.copy(dist[:, k * 512:(k + 1) * 512], ps[:])

                    cand = smp.tile([128, NCAND], f32, tag="cand")
                    lidx = smp.tile([128, NCAND], u16, tag="lidx")
                    for blk in range(NBLK):
                        dslice = dist[:, blk * 128:(blk + 1) * 128]
                        nc.vector.max(out=cand[:, blk * 8:blk * 8 + 8], in_=dslice)
                        nc.vector.max_index(out=lidx[:, blk * 8:blk * 8 + 8],
                                            in_max=cand[:, blk * 8:blk * 8 + 8],
                                            in_values=dslice)
                    gidx_cand = smp.tile([128, NCAND], u16, tag="gcand")
                    nc.vector.tensor_tensor(out=gidx_cand[:], in0=lidx[:],
                                            in1=blockbase[:],
                                            op=mybir.AluOpType.add)

                    # exact top-32 of the 512 candidates
                    work = smp.tile([128, NCAND], f32, tag="work")
                    mxc = smp.tile([128, K], f32, tag="mxc")
                    pos = smp.tile([128, K], u16, tag="pos")
                    src = cand
                    for it in range(4):
                        nc.vector.max(out=mxc[:, it * 8:it * 8 + 8], in_=src[:])
                        nc.vector.max_index(out=pos[:, it * 8:it * 8 + 8],
                                            in_max=mxc[:, it * 8:it * 8 + 8],
                                            in_values=src[:])
                        if it < 3:
                            nc.vector.match_replace(
                                out=work[:], in_to_replace=mxc[:, it * 8:it * 8 + 8],
                                in_values=src[:], imm_value=NEG)
                            src = work

                    # extract global idx at the 32 positions via two local_scatters
                    rank_at = smp.tile([128, NCAND], u16, tag="rank_at")
                    nc.gpsimd.local_scatter(
                        out_ap=rank_at[:], data_ap=ranks[:],
                        idxs_ap=pos[:].bitcast(mybir.dt.int16),
                        channels=128, num_elems=NCAND, num_idxs=K)
                    rankm1 = smp.tile([128, NCAND], mybir.dt.int16, tag="rankm1")
                    nc.vector.tensor_scalar(rankm1[:], rank_at[:], 1.0, None,
                                            op0=mybir.AluOpType.subtract)
                    gidx = smp.tile([128, K], u16, tag="gidx")
                    ls = nc.gpsimd.local_scatter(
                        out_ap=gidx[:], data_ap=gidx_cand[:], idxs_ap=rankm1[:],
                        channels=128, num_elems=K, num_idxs=NCAND)
                    last_ls = ls

                    # ball-query mask: slots with dist > R^2 (ndist < -R^2) -> idx0
                    mask = smp.tile([128, K], u32, tag="mask")
                    nc.vector.tensor_scalar(mask[:], mxc[:], -R2, None,
                                            op0=mybir.AluOpType.is_lt)
                    nc.vector.copy_predicated(gidx[:], mask[:],
                                              gidx[:, 0:1].to_broadcast([128, K]))
                    nc.sync.dma_start(
                        out=idx_dram[:].rearrange("(t s p) -> t p s", t=NT, s=K)[t],
                        in_=gidx[:])
                    if debug and t == 0:
                        nc.sync.dma_start(out=dbg["d_dist"][:], in_=dist[:])
                        nc.sync.dma_start(out=dbg["d_cand"][:], in_=cand[:])
                        nc.sync.dma_start(out=dbg["d_gcand"][:], in_=gidx_cand[:])
                        nc.sync.dma_start(out=dbg["d_mxc"][:], in_=mxc[:])
                        nc.sync.dma_start(out=dbg["d_pos"][:], in_=pos[:])
                        nc.sync.dma_start(out=dbg["d_gidx"][:], in_=gidx[:])

            # ---------------- phase B: gather + reduce ----------------
            with tc.tile_pool(name="h_ps", bufs=2, space="PSUM") as hps, \
                 tc.tile_pool(name="wrap", bufs=1) as wp2, \
                 tc.tile_pool(name="gath", bufs=2) as gp:
                # wrapped idx tile for all tiles, replicated into each
                # 16-partition group (one DMA per group)
                idxw_all = wp2.tile([128, NT * (K * 128 // 16)], u16, tag="idxw")
                for r in range(8):
                    nc.sync.dma_start(
                        out=idxw_all[r * 16:(r + 1) * 16, :].rearrange(
                            "w (t j) -> w t j", t=NT),
                        in_=idx_dram[:].rearrange("(t j w) -> w t j", t=NT, w=16))
                for t in range(NT):
                    q0 = t * 128
                    gath = gp.tile([128, K * C], f32, tag="gath")
                    dg = nc.gpsimd.dma_gather(
                        out_ap=gath[:].rearrange("p (s c) -> p s c", s=K),
                        in_ap=gt_dram[:],
                        idxs_ap=idxw_all[:, t * 256:(t + 1) * 256].bitcast(mybir.dt.int16),
                        num_idxs=K * 128, num_idxs_reg=K * 128, elem_size=C,
                        single_packet=False)
                    if last_ls is not None:
                        tile.add_dep_helper(
                            dg.ins, last_ls.ins, sync=False,
                            reason="keep mlp-library pool ops after local_scatter ops")

                    hp = hps.tile([128, C], f32, tag="hps")
                    nc.tensor.matmul(hp[:], a4[:, q0:q0 + 128], rhs4[:],
                                     start=True, stop=True)
                    ht = wp2.tile([128, C], f32, tag="ht")
                    nc.scalar.copy(ht[:], hp[:])

                    gmax = wp2.tile([128, C], f32, tag="gmax")
                    nc.vector.reduce_max(
                        out=gmax[:],
                        in_=gath[:].rearrange("p (s c) -> p c s", s=K),
                        axis=mybir.AxisListType.X)
                    o = wp2.tile([128, C], f32, tag="o")
                    nc.vector.tensor_sub(o[:], gmax[:], ht[:])
                    nc.vector.tensor_scalar_max(o[:], o[:], 0.0)
                    nc.sync.dma_start(out=y_out[q0:q0 + 128, :], in_=o[:])
                    if debug and t == 0:
                        nc.sync.dma_start(out=dbg["d_gath"][:], in_=gath[:])
                if debug:
                    nc.sync.dma_start(out=dbg["d_gt"][:], in_=gt_dram[:])

    nc.compile()
    return nc


def _get_nc():
    if "nc" not in _CACHE:
        _CACHE["nc"] = _build()
    return _CACHE["nc"]


def _make_in_maps(inputs):
    points_coor = np.ascontiguousarray(inputs["points_coor"], np.float32)
    points_fea = np.ascontiguousarray(inputs["points_fea"], np.float32)
    W = np.ascontiguousarray(inputs["W"], np.float32)
    bnt = np.ascontiguousarray(
        np.stack([inputs["gamma"], inputs["beta"], inputs["running_mean"],
                  inputs["running_var"]], axis=1), np.float32)
    blockbase = np.repeat((np.arange(NBLK, dtype=np.uint16) * 128), 8)
    blockbase = np.tile(blockbase[None, :], (128, 1)).copy()
    ranks = np.tile(np.arange(1, K + 1, dtype=np.uint16)[None, :], (128, 1)).copy()
    constrows = np.stack([np.ones(N, np.float32), -np.ones(N, np.float32)])
    in_maps = []
    for core in range(NCORES):
        b, h = core // 2, core % 2
        in_maps.append(dict(
            xyz=points_coor[b],
            xyzq=np.ascontiguousarray(points_coor[b][:, h * QPC:(h + 1) * QPC]),
            fea=points_fea[b],
            w=W,
            bnt=bnt,
            blockbase=blockbase,
            ranks=ranks,
            constrows=constrows,
        ))
    return in_maps


def kernel(points_coor, points_fea, W, gamma, beta, running_mean, running_var,
           **_unused):
    inputs = dict(points_coor=points_coor, points_fea=points_fea, W=W,
                  gamma=gamma, beta=beta, running_mean=running_mean,
                  running_var=running_var)
    nc = _get_nc()
    in_maps = _make_in_maps(inputs)
    res = run_bass_kernel_spmd(nc, in_maps, list(range(NCORES)))
    out = np.empty((B, C, N), np.float32)
    for core in range(NCORES):
        b, h = core // 2, core % 2
        out[b, :, h * QPC:(h + 1) * QPC] = res.results[core]["y"].T
    return out



# revision 4
# speedup vs baseline: 1.0066x; 1.0066x over previous
"""Trainium2 Bass kernel for nn_LocalAggregation (ball-query KNN + grouped MLP + max-pool).

Math refactor: with BN in eval mode and ReLU/max commuting past the per-query
affine part, the whole conv+BN+ReLU+max collapses to
    out[c, m] = relu( max_{k in NN32(m)} Gt[k, c]  -  Ht[m, c] )
where Gt = (diag(s)@W) @ [fea; xyz/R]  per point, Ht per query,
s = gamma/sqrt(var+eps), and the ball-query mask replaces far neighbors with
the nearest (self) index.

Selection: negated distances nd[m,n] = 2q.k - sq_m - sq_n come straight from
one fp32r matmul per 512-key chunk; max8/max_index scan the PSUM chunk
directly (top-8 per 512 keys -> 128 candidates), then 4 rounds of
max8/match_replace give the exact top-32 of the candidates.

Sharding: 8 cores = 4 batches x 2 query-halves. Each core handles 4096
queries x all 8192 keys of one batch sample.
"""

import numpy as np

import concourse.bacc as bacc
import concourse.mybir as mybir
from concourse import tile
from concourse.bass_utils import run_bass_kernel_spmd

B, C, N = 4, 64, 8192
K = 32
RADIUS = 0.2
R2 = RADIUS * RADIUS
EPS = 1e-5
CIN = C + 3            # 67
NCORES = 8
QPC = N // 2           # queries per core
NT = QPC // 128        # query tiles per core (32)
NBLK = N // 512        # key blocks per tile (16)
NCAND = NBLK * 8       # candidates per query (128)
NEG = -3.0e38

f32 = mybir.dt.float32
f32r = mybir.dt.float32r
u16 = mybir.dt.uint16
u32 = mybir.dt.uint32

_CACHE = {}


def _build(debug=False):
    nc = bacc.Bacc("TRN2", target_bir_lowering=False, debug=False,
                   num_devices=NCORES)

    xyz_in = nc.dram_tensor("xyz", [3, N], f32, kind="ExternalInput").ap()
    xyzq_in = nc.dram_tensor("xyzq", [3, QPC], f32, kind="ExternalInput").ap()
    fea_in = nc.dram_tensor("fea", [C, N], f32, kind="ExternalInput").ap()
    w_in = nc.dram_tensor("w", [C, CIN], f32, kind="ExternalInput").ap()
    bnt_in = nc.dram_tensor("bnt", [C, 4], f32, kind="ExternalInput").ap()
    blockbase_in = nc.dram_tensor("blockbase", [128, NCAND], u16,
                                  kind="ExternalInput").ap()
    ranks_in = nc.dram_tensor("ranks", [128, K], u16,
                              kind="ExternalInput").ap()
    constrows_in = nc.dram_tensor("constrows", [2, N], f32,
                                  kind="ExternalInput").ap()
    y_out = nc.dram_tensor("y", [QPC, C], f32, kind="ExternalOutput").ap()

    nsq_dram = nc.dram_tensor("nsq_scr", [1, N], f32).ap()
    sqq_dram = nc.dram_tensor("sqq_scr", [1, QPC], f32).ap()
    cc_dram = nc.dram_tensor("cc_scr", [C, 1], f32).ap()
    wct_dram = nc.dram_tensor("wct_scr", [3, C], f32).ap()
    gt_dram = nc.dram_tensor("gt", [N, C], f32).ap()
    idx_dram = nc.dram_tensor("idxb", [NT * K * 128], u16).ap()

    with tile.TileContext(nc) as tc:
        # ---------------- persistent tiles ----------------
        with tc.tile_pool(name="persist", bufs=1) as pp:
            a5 = pp.tile([5, QPC], f32, tag="a5")        # [x,y,z,1,sq] queries
            b5 = pp.tile([5, N], f32, tag="b5")          # [2x,2y,2z,-sq,-1] keys
            rhs4 = pp.tile([4, C], f32, tag="rhs4")      # [Wc'/R ; cc] rhs
            blockbase = pp.tile([128, NCAND], u16, tag="bb")
            ranks = pp.tile([128, K], u16, tag="ranks")
            nc.sync.dma_start(out=blockbase[:], in_=blockbase_in[:])
            nc.sync.dma_start(out=ranks[:], in_=ranks_in[:])

            # ---------------- prep ----------------
            with tc.tile_pool(name="prep", bufs=1) as sp, \
                 tc.tile_pool(name="prep_ps", bufs=2, space="PSUM") as pps:
                f67 = sp.tile([CIN, N], f32)
                w = sp.tile([C, CIN], f32)
                bnt = sp.tile([C, 4], f32)
                nc.sync.dma_start(out=f67[:C, :], in_=fea_in[:])
                nc.sync.dma_start(out=f67[C:, :], in_=xyz_in[:])
                nc.sync.dma_start(out=w[:], in_=w_in[:])
                nc.sync.dma_start(out=bnt[:], in_=bnt_in[:])
                nc.sync.dma_start(out=a5[0:3, :], in_=xyzq_in[:])

                # s = gamma / sqrt(var + eps); cc = s*mean - beta (per channel)
                s_t = sp.tile([C, 1], f32)
                tmp = sp.tile([C, 1], f32)
                nc.vector.tensor_scalar_add(tmp[:], bnt[:, 3:4], EPS)
                nc.scalar.activation(tmp[:], tmp[:],
                                     mybir.ActivationFunctionType.Sqrt)
                nc.vector.reciprocal(tmp[:], tmp[:])
                nc.vector.tensor_mul(s_t[:], bnt[:, 0:1], tmp[:])
                cc_t = sp.tile([C, 1], f32)
                nc.vector.tensor_mul(cc_t[:], bnt[:, 2:3], s_t[:])
                nc.vector.tensor_sub(cc_t[:], cc_t[:], bnt[:, 1:2])
                nc.sync.dma_start(out=cc_dram[:], in_=cc_t[:])

                # W' = diag(s) @ W ; coor columns additionally * (1/R)
                wp = sp.tile([C, CIN], f32)
                nc.vector.tensor_scalar_mul(wp[:], w[:], s_t[:])
                nc.vector.tensor_scalar_mul(wp[:, C:], wp[:, C:], 1.0 / RADIUS)

                # W'T[k, c] via PE transpose against an identity matrix
                diag = sp.tile([C, C], f32)
                nc.gpsimd.memset(diag[:], 0.0)
                one_col = sp.tile([C, 1], f32)
                nc.gpsimd.memset(one_col[:], 1.0)
                nc.gpsimd.affine_select(
                    diag[:], one_col[:].to_broadcast([C, C]),
                    pattern=[[-1, C]], base=0, channel_multiplier=1,
                    compare_op=mybir.AluOpType.is_equal, fill=0.0)
                wpt_ps = pps.tile([CIN, C], f32)
                nc.tensor.matmul(wpt_ps[:], wp[:], diag[:], start=True, stop=True)
                wpt = sp.tile([CIN, C], f32)
                nc.scalar.copy(wpt[:], wpt_ps[:])
                # stash coor rows of W'T for rhs4 assembly (partition shift via DRAM)
                nc.sync.dma_start(out=wct_dram[:], in_=wpt[C:, :])

                # -sq over keys, sq over queries (PE ones-reduction, chunked)
                ones3 = sp.tile([3, 1], f32)
                nc.gpsimd.memset(ones3[:], 1.0)
                for k in range(N // 512):
                    t3 = sp.tile([3, 512], f32, tag="t3")
                    nc.vector.tensor_mul(t3[:], f67[C:, k * 512:(k + 1) * 512],
                                         f67[C:, k * 512:(k + 1) * 512])
                    ps = pps.tile([1, 512], f32, tag="sqps")
                    nc.tensor.matmul(ps[:], ones3[:], t3[:], start=True, stop=True)
                    nsqc = sp.tile([1, 512], f32, tag="nsqc")
                    nc.vector.tensor_scalar_mul(nsqc[:], ps[:], -1.0)
                    nc.sync.dma_start(out=nsq_dram[:, k * 512:(k + 1) * 512],
                                      in_=nsqc[:])
                for k in range(QPC // 512):
                    t3 = sp.tile([3, 512], f32, tag="t3")
                    nc.vector.tensor_mul(t3[:], a5[0:3, k * 512:(k + 1) * 512],
                                         a5[0:3, k * 512:(k + 1) * 512])
                    ps = pps.tile([1, 512], f32, tag="sqps")
                    nc.tensor.matmul(ps[:], ones3[:], t3[:], start=True, stop=True)
                    sqc = sp.tile([1, 512], f32, tag="sqc")
                    nc.scalar.copy(sqc[:], ps[:])
                    nc.sync.dma_start(out=sqq_dram[:, k * 512:(k + 1) * 512],
                                      in_=sqc[:])

                # B5 = [2x, 2y, 2z, -sq, -1] over keys
                nc.sync.dma_start(out=b5[0:3, :], in_=xyz_in[:])
                nc.vector.tensor_scalar_mul(b5[0:3, :], b5[0:3, :], 2.0)
                nc.sync.dma_start(out=b5[3:4, :], in_=nsq_dram[:])
                nc.sync.dma_start(out=b5[4:5, :], in_=constrows_in[1:2, :])

                # A5 rows: [x, y, z, 1, sq] over queries
                nc.sync.dma_start(out=a5[3:4, :], in_=constrows_in[0:1, :QPC])
                nc.sync.dma_start(out=a5[4:5, :], in_=sqq_dram[:])

                # rhs4 = [W'T coor rows ; ccT]
                nc.sync.dma_start(out=rhs4[0:3, :], in_=wct_dram[:])
                nc.sync.dma_start(out=rhs4[3:4, :],
                                  in_=cc_dram[:].rearrange("c one -> one c"))

                # Gt[n, c] = sum_p F67[p, n] * W'T[p, c]  -> DRAM [N, C]
                # 8 matmuls of 64 cols share one PSUM bank -> 1 copy per bank
                gstage = sp.tile([128, (N // 128) * C], f32)
                for j in range(N // 1024):
                    gps = pps.tile([128, 512], f32, tag="gps")
                    for i in range(8):
                        blk = j * 8 + i
                        nc.tensor.matmul(gps[:, i * C:(i + 1) * C],
                                         f67[:, blk * 128:(blk + 1) * 128],
                                         wpt[:], start=True, stop=True)
                    nc.scalar.copy(gstage[:, j * 8 * C:(j + 1) * 8 * C], gps[:])
                nc.sync.dma_start(
                    out=gt_dram[:].rearrange("(blk p) c -> p blk c", p=128),
                    in_=gstage[:].rearrange("p (blk c) -> p blk c", c=C))

            # ---------------- phase A: selection over query tiles ----------------
            last_ls = None
            with tc.tile_pool(name="nd_ps", bufs=6, space="PSUM") as ndp, \
                 tc.tile_pool(name="small", bufs=2) as smp:
                for t in range(NT):
                    q0 = t * 128
                    cand = smp.tile([128, NCAND], f32, tag="cand")
                    lidx = smp.tile([128, NCAND], u16, tag="lidx")
                    for k in range(NBLK):
                        ps = ndp.tile([128, 512], f32, tag="nd")
                        nc.tensor.matmul(ps[:],
                                         a5[:, q0:q0 + 128],
                                         b5[:, k * 512:(k + 1) * 512],
                                         start=True, stop=True)
                        nc.vector.max(out=cand[:, k * 8:k * 8 + 8], in_=ps[:])
                        nc.vector.max_index(out=lidx[:, k * 8:k * 8 + 8],
                                            in_max=cand[:, k * 8:k * 8 + 8],
                                            in_values=ps[:])
                    gidx_cand = smp.tile([128, NCAND], u16, tag="gcand")
                    nc.vector.tensor_tensor(out=gidx_cand[:], in0=lidx[:],
                                            in1=blockbase[:],
                                            op=mybir.AluOpType.add)

                    # exact top-32 of the 128 candidates
                    work = smp.tile([128, NCAND], f32, tag="work")
                    mxc = smp.tile([128, K], f32, tag="mxc")
                    pos = smp.tile([128, K], u16, tag="pos")
                    src = cand
                    for it in range(4):
                        nc.vector.max(out=mxc[:, it * 8:it * 8 + 8], in_=src[:])
                        nc.vector.max_index(out=pos[:, it * 8:it * 8 + 8],
                                            in_max=mxc[:, it * 8:it * 8 + 8],
                                            in_values=src[:])
                        if it < 3:
                            nc.vector.match_replace(
                                out=work[:], in_to_replace=mxc[:, it * 8:it * 8 + 8],
                                in_values=src[:], imm_value=NEG)
                            src = work

                    # extract global idx at the 32 positions via two local_scatters
                    rank_at = smp.tile([128, NCAND], u16, tag="rank_at")
                    nc.gpsimd.local_scatter(
                        out_ap=rank_at[:], data_ap=ranks[:],
                        idxs_ap=pos[:].bitcast(mybir.dt.int16),
                        channels=128, num_elems=NCAND, num_idxs=K)
                    rankm1 = smp.tile([128, NCAND], mybir.dt.int16, tag="rankm1")
                    nc.vector.tensor_scalar(rankm1[:], rank_at[:], 1.0, None,
                                            op0=mybir.AluOpType.subtract)
                    gidx = smp.tile([128, K], u16, tag="gidx")
                    ls = nc.gpsimd.local_scatter(
                        out_ap=gidx[:], data_ap=gidx_cand[:], idxs_ap=rankm1[:],
                        channels=128, num_elems=K, num_idxs=NCAND)
                    last_ls = ls

                    # ball-query mask: slots with dist > R^2 (ndist < sq_m - R^2) -> idx0
                    mask = smp.tile([128, K], u32, tag="mask")
                    nc.vector.tensor_scalar(mask[:], mxc[:], -R2, None,
                                            op0=mybir.AluOpType.is_lt)
                    nc.vector.copy_predicated(gidx[:], mask[:],
                                              gidx[:, 0:1].to_broadcast([128, K]))
                    nc.sync.dma_start(
                        out=idx_dram[:].rearrange("(t s p) -> t p s", t=NT, s=K)[t],
                        in_=gidx[:])

            # ---------------- phase B: gather + reduce ----------------
            with tc.tile_pool(name="h_ps", bufs=2, space="PSUM") as hps, \
                 tc.tile_pool(name="wrap", bufs=1) as wp2, \
                 tc.tile_pool(name="gath", bufs=2) as gp:
                # wrapped idx tile for all tiles, replicated into each
                # 16-partition group (one DMA per group)
                idxw_all = wp2.tile([128, NT * (K * 128 // 16)], u16, tag="idxw")
                for r in range(8):
                    nc.sync.dma_start(
                        out=idxw_all[r * 16:(r + 1) * 16, :].rearrange(
                            "w (t j) -> w t j", t=NT),
                        in_=idx_dram[:].rearrange("(t j w) -> w t j", t=NT, w=16))
                for t in range(NT):
                    q0 = t * 128
                    gath = gp.tile([128, K * C], f32, tag="gath")
                    dg = nc.gpsimd.dma_gather(
                        out_ap=gath[:].rearrange("p (s c) -> p s c", s=K),
                        in_ap=gt_dram[:],
                        idxs_ap=idxw_all[:, t * 256:(t + 1) * 256].bitcast(mybir.dt.int16),
                        num_idxs=K * 128, num_idxs_reg=K * 128, elem_size=C,
                        single_packet=False)
                    if last_ls is not None:
                        tile.add_dep_helper(
                            dg.ins, last_ls.ins, sync=False,
                            reason="keep mlp-library pool ops after local_scatter ops")

                    hp = hps.tile([128, C], f32, tag="hps")
                    nc.tensor.matmul(hp[:], a5[0:4, q0:q0 + 128], rhs4[:],
                                     start=True, stop=True)
                    ht = wp2.tile([128, C], f32, tag="ht")
                    nc.scalar.copy(ht[:], hp[:])

                    gmax = wp2.tile([128, C], f32, tag="gmax")
                    nc.vector.reduce_max(
                        out=gmax[:],
                        in_=gath[:].rearrange("p (s c) -> p c s", s=K),
                        axis=mybir.AxisListType.X)
                    o = wp2.tile([128, C], f32, tag="o")
                    nc.vector.tensor_sub(o[:], gmax[:], ht[:])
                    nc.vector.tensor_scalar_max(o[:], o[:], 0.0)
                    nc.sync.dma_start(out=y_out[q0:q0 + 128, :], in_=o[:])

    nc.compile()
    return nc


def _get_nc():
    if "nc" not in _CACHE:
        _CACHE["nc"] = _build()
    return _CACHE["nc"]


def _make_in_maps(inputs):
    points_coor = np.ascontiguousarray(inputs["points_coor"], np.float32)
    points_fea = np.ascontiguousarray(inputs["points_fea"], np.float32)
    W = np.ascontiguousarray(inputs["W"], np.float32)
    bnt = np.ascontiguousarray(
        np.stack([inputs["gamma"], inputs["beta"], inputs["running_mean"],
                  inputs["running_var"]], axis=1), np.float32)
    blockbase = np.repeat((np.arange(NBLK, dtype=np.uint16) * 512), 8)
    blockbase = np.tile(blockbase[None, :], (128, 1)).copy()
    ranks = np.tile(np.arange(1, K + 1, dtype=np.uint16)[None, :], (128, 1)).copy()
    constrows = np.stack([np.ones(N, np.float32), -np.ones(N, np.float32)])
    in_maps = []
    for core in range(NCORES):
        b, h = core // 2, core % 2
        in_maps.append(dict(
            xyz=points_coor[b],
            xyzq=np.ascontiguousarray(points_coor[b][:, h * QPC:(h + 1) * QPC]),
            fea=points_fea[b],
            w=W,
            bnt=bnt,
            blockbase=blockbase,
            ranks=ranks,
            constrows=constrows,
        ))
    return in_maps


def kernel(points_coor, points_fea, W, gamma, beta, running_mean, running_var,
           **_unused):
    inputs = dict(points_coor=points_coor, points_fea=points_fea, W=W,
                  gamma=gamma, beta=beta, running_mean=running_mean,
                  running_var=running_var)
    nc = _get_nc()
    in_maps = _make_in_maps(inputs)
    res = run_bass_kernel_spmd(nc, in_maps, list(range(NCORES)))
    out = np.empty((B, C, N), np.float32)
    for core in range(NCORES):
        b, h = core // 2, core % 2
        out[b, :, h * QPC:(h + 1) * QPC] = res.results[core]["y"].T
    return out


# revision 15
# speedup vs baseline: 1.2167x; 1.2088x over previous
"""Trainium2 Bass kernel for nn_LocalAggregation (ball-query KNN + grouped MLP + max-pool).

Math refactor: with BN in eval mode and ReLU/max commuting past the per-query
affine part, the whole conv+BN+ReLU+max collapses to
    out[c, m] = relu( max_{k in NN32(m)} GtT[c, k]  -  HtT[c, m] )
where GtT = (diag(s)@W) @ [fea; xyz/R] per point (channel-major), HtT per
query, s = gamma/sqrt(var+eps); the ball-query mask replaces far neighbors
with the top-1 (in-ball) index.

Selection: negated distances nd[m,n] = 2q.k - sq_m - sq_n come from one fp32
matmul per 512-key chunk; max8/max_index scan the PSUM chunk directly (top-8
per 512 keys -> 128 exact-fp32 candidates), 4 rounds of max8/match_replace
give the exact top-32, and two gpsimd local_scatters extract the global key
ids in rank order.

Gather: GPSIMD ap_gather from a channel-major SBUF copy of GtT duplicated
across both partition halves (each Q7 core gathers 2048 of the 4096
query-slot keys); output is channel-major so y needs no host transpose.
Tiles run in groups of GRP: selections (local_scatter lib) for the whole
group, then gathers (ap_gather lib), limiting Q7 IRAM library reloads while
gathers overlap the next group's DVE/PE work.

Sharding: 8 cores = 4 batches x 2 query-halves. Each core handles 4096
queries x all 8192 keys of one batch sample.
"""

import numpy as np

import concourse.bacc as bacc
import concourse.mybir as mybir
from concourse import tile
from concourse.bass_utils import run_bass_kernel_spmd

B, C, N = 4, 64, 8192
K = 32
RADIUS = 0.2
R2 = RADIUS * RADIUS
EPS = 1e-5
CIN = C + 3            # 67
NCORES = 8
QPC = N // 2           # queries per core
NT = QPC // 128        # query tiles per core (32)
NBLK = N // 512        # key chunks per tile (16)
NCAND = NBLK * 8       # candidates per query (128)
GRP = 8                # tiles per pipeline group (library-reload batching)
NEG = -3.0e38

f32 = mybir.dt.float32
u16 = mybir.dt.uint16
u32 = mybir.dt.uint32
i16 = mybir.dt.int16

_CACHE = {}


def _build():
    nc = bacc.Bacc("TRN2", target_bir_lowering=False, debug=False,
                   num_devices=NCORES)

    xyz_in = nc.dram_tensor("xyz", [3, N], f32, kind="ExternalInput").ap()
    xyzq_in = nc.dram_tensor("xyzq", [3, QPC], f32, kind="ExternalInput").ap()
    fea_in = nc.dram_tensor("fea", [C, N], f32, kind="ExternalInput").ap()
    w_in = nc.dram_tensor("w", [C, CIN], f32, kind="ExternalInput").ap()
    bnt_in = nc.dram_tensor("bnt", [C, 4], f32, kind="ExternalInput").ap()
    blockbase_in = nc.dram_tensor("blockbase", [128, NCAND], u16,
                                  kind="ExternalInput").ap()
    ranks_in = nc.dram_tensor("ranks", [128, K], u16,
                              kind="ExternalInput").ap()
    constrows_in = nc.dram_tensor("constrows", [2, N], f32,
                                  kind="ExternalInput").ap()
    y_out = nc.dram_tensor("y", [C, QPC], f32, kind="ExternalOutput").ap()

    nsq_dram = nc.dram_tensor("nsq_scr", [1, N], f32).ap()
    sqq_dram = nc.dram_tensor("sqq_scr", [1, QPC], f32).ap()
    cc_dram = nc.dram_tensor("cc_scr", [C, 1], f32).ap()
    wct_dram = nc.dram_tensor("wct_scr", [3, C], f32).ap()
    idx_dram = nc.dram_tensor("idxb", [NT * 2 * 2048], u16).ap()

    with tile.TileContext(nc) as tc:
        # ---------------- persistent tiles ----------------
        with tc.tile_pool(name="persist", bufs=1) as pp:
            a5 = pp.tile([5, QPC], f32, tag="a5")        # [x,y,z,1,sq] queries
            b5 = pp.tile([5, N], f32, tag="b5")          # [2x,2y,2z,-sq,-1] keys
            a8 = pp.tile([8, NT * 64], f32, tag="a8")    # stacked query halves
            lhsT8 = pp.tile([8, 128], f32, tag="lhsT8")  # block-diag rhs4 pair
            gtt = pp.tile([128, N], f32, tag="gtt")      # GtT dup both halves
            blockbase = pp.tile([128, NCAND], u16, tag="bb")
            ranks = pp.tile([128, K], u16, tag="ranks")
            nc.sync.dma_start(out=blockbase[:], in_=blockbase_in[:])
            nc.sync.dma_start(out=ranks[:], in_=ranks_in[:])

            # ---------------- prep ----------------
            with tc.tile_pool(name="prep", bufs=1) as sp, \
                 tc.tile_pool(name="prep_ps", bufs=2, space="PSUM") as pps:
                f67 = sp.tile([CIN, N], f32)
                w = sp.tile([C, CIN], f32)
                bnt = sp.tile([C, 4], f32)
                nc.sync.dma_start(out=f67[:C, :], in_=fea_in[:])
                nc.sync.dma_start(out=f67[C:, :], in_=xyz_in[:])
                nc.sync.dma_start(out=w[:], in_=w_in[:])
                nc.sync.dma_start(out=bnt[:], in_=bnt_in[:])
                nc.sync.dma_start(out=a5[0:3, :], in_=xyzq_in[:])

                # s = gamma / sqrt(var + eps); cc = s*mean - beta (per channel)
                s_t = sp.tile([C, 1], f32)
                tmp = sp.tile([C, 1], f32)
                nc.vector.tensor_scalar_add(tmp[:], bnt[:, 3:4], EPS)
                nc.scalar.activation(tmp[:], tmp[:],
                                     mybir.ActivationFunctionType.Sqrt)
                nc.vector.reciprocal(tmp[:], tmp[:])
                nc.vector.tensor_mul(s_t[:], bnt[:, 0:1], tmp[:])
                cc_t = sp.tile([C, 1], f32)
                nc.vector.tensor_mul(cc_t[:], bnt[:, 2:3], s_t[:])
                nc.vector.tensor_sub(cc_t[:], cc_t[:], bnt[:, 1:2])
                nc.sync.dma_start(out=cc_dram[:], in_=cc_t[:])

                # W' = diag(s) @ W ; coor columns additionally * (1/R)
                wp = sp.tile([C, CIN], f32)
                nc.vector.tensor_scalar_mul(wp[:], w[:], s_t[:])
                nc.vector.tensor_scalar_mul(wp[:, C:], wp[:, C:], 1.0 / RADIUS)

                # W'T[k, c] via PE transpose against an identity matrix
                diag = sp.tile([C, C], f32)
                nc.gpsimd.memset(diag[:], 0.0)
                one_col = sp.tile([C, 1], f32)
                nc.gpsimd.memset(one_col[:], 1.0)
                nc.gpsimd.affine_select(
                    diag[:], one_col[:].to_broadcast([C, C]),
                    pattern=[[-1, C]], base=0, channel_multiplier=1,
                    compare_op=mybir.AluOpType.is_equal, fill=0.0)
                wpt_ps = pps.tile([CIN, C], f32)
                nc.tensor.matmul(wpt_ps[:], wp[:], diag[:], start=True, stop=True)
                wpt = sp.tile([CIN, C], f32)
                nc.scalar.copy(wpt[:], wpt_ps[:])
                nc.sync.dma_start(out=wct_dram[:], in_=wpt[C:, :])

                # -sq over keys, sq over queries (PE ones-reduction, chunked)
                ones3 = sp.tile([3, 1], f32)
                nc.gpsimd.memset(ones3[:], 1.0)
                for k in range(N // 512):
                    t3 = sp.tile([3, 512], f32, tag="t3")
                    nc.vector.tensor_mul(t3[:], f67[C:, k * 512:(k + 1) * 512],
                                         f67[C:, k * 512:(k + 1) * 512])
                    ps = pps.tile([1, 512], f32, tag="sqps")
                    nc.tensor.matmul(ps[:], ones3[:], t3[:], start=True, stop=True)
                    nsqc = sp.tile([1, 512], f32, tag="nsqc")
                    nc.vector.tensor_scalar_mul(nsqc[:], ps[:], -1.0)
                    nc.sync.dma_start(out=nsq_dram[:, k * 512:(k + 1) * 512],
                                      in_=nsqc[:])
                for k in range(QPC // 512):
                    t3 = sp.tile([3, 512], f32, tag="t3")
                    nc.vector.tensor_mul(t3[:], a5[0:3, k * 512:(k + 1) * 512],
                                         a5[0:3, k * 512:(k + 1) * 512])
                    ps = pps.tile([1, 512], f32, tag="sqps")
                    nc.tensor.matmul(ps[:], ones3[:], t3[:], start=True, stop=True)
                    sqc = sp.tile([1, 512], f32, tag="sqc")
                    nc.scalar.copy(sqc[:], ps[:])
                    nc.sync.dma_start(out=sqq_dram[:, k * 512:(k + 1) * 512],
                                      in_=sqc[:])

                # B5 = [2x, 2y, 2z, -sq, -1] over keys
                nc.sync.dma_start(out=b5[0:3, :], in_=xyz_in[:])
                nc.vector.tensor_scalar_mul(b5[0:3, :], b5[0:3, :], 2.0)
                nc.sync.dma_start(out=b5[3:4, :], in_=nsq_dram[:])
                nc.sync.dma_start(out=b5[4:5, :], in_=constrows_in[1:2, :])

                # A5 rows: [x, y, z, 1, sq] over queries
                nc.sync.dma_start(out=a5[3:4, :], in_=constrows_in[0:1, :QPC])
                nc.sync.dma_start(out=a5[4:5, :], in_=sqq_dram[:])

                # a8: stacked [x,y,z,1] for query halves (64-col per tile)
                xyzq_v = xyzq_in.rearrange("c (t g q) -> c t g q", g=2, q=64)
                nc.sync.dma_start(
                    out=a8[0:3, :].rearrange("c (t q) -> c t q", q=64),
                    in_=xyzq_v[:, :, 0, :])
                nc.sync.dma_start(
                    out=a8[4:7, :].rearrange("c (t q) -> c t q", q=64),
                    in_=xyzq_v[:, :, 1, :])
                nc.sync.dma_start(out=a8[3:4, :], in_=constrows_in[0:1, :NT * 64])
                nc.sync.dma_start(out=a8[7:8, :], in_=constrows_in[0:1, :NT * 64])

                # lhsT8: block diag [rhs4 ; rhs4] (rhs4 = [W'T coor rows; ccT])
                nc.gpsimd.memset(lhsT8[:], 0.0)
                nc.sync.dma_start(out=lhsT8[0:3, 0:C], in_=wct_dram[:])
                nc.sync.dma_start(out=lhsT8[3:4, 0:C],
                                  in_=cc_dram[:].rearrange("c one -> one c"))
                nc.sync.dma_start(out=lhsT8[4:7, C:2 * C], in_=wct_dram[:])
                nc.sync.dma_start(out=lhsT8[7:8, C:2 * C],
                                  in_=cc_dram[:].rearrange("c one -> one c"))

                # GtT[c, n] channel-major; duplicate into partitions 64-127
                for j in range(N // 512):
                    gps = pps.tile([C, 512], f32, tag="gps")
                    nc.tensor.matmul(gps[:], wpt[:],
                                     f67[:, j * 512:(j + 1) * 512],
                                     start=True, stop=True)
                    nc.scalar.copy(gtt[0:C, j * 512:(j + 1) * 512], gps[:])
                nc.sync.dma_start(out=gtt[C:2 * C, :], in_=gtt[0:C, :])

            # -------- grouped per-tile pipeline: select then gather --------
            idx_v = idx_dram.rearrange(
                "(t h lo q hi) -> t h q hi lo", t=NT, h=2, lo=16, q=64, hi=2)
            idx_l = idx_dram.rearrange(
                "(t h lo c) -> t h lo c", t=NT, h=2, lo=16, c=128)
            with tc.tile_pool(name="nd_ps", bufs=6, space="PSUM") as ndp, \
                 tc.tile_pool(name="h_ps", bufs=2, space="PSUM") as hps, \
                 tc.tile_pool(name="small", bufs=2) as smp, \
                 tc.tile_pool(name="idxp", bufs=GRP + 2) as ixp, \
                 tc.tile_pool(name="gath", bufs=2) as gp:
                for g0 in range(0, NT, GRP):
                    # ---- selection for the group ----
                    for t in range(g0, g0 + GRP):
                        q0 = t * 128
                        cand = smp.tile([128, NCAND], f32, tag="cand")
                        lidx = smp.tile([128, NCAND], u16, tag="lidx")
                        for k in range(NBLK):
                            ps = ndp.tile([128, 512], f32, tag="nd")
                            nc.tensor.matmul(ps[:], a5[:, q0:q0 + 128],
                                             b5[:, k * 512:(k + 1) * 512],
                                             start=True, stop=True)
                            nc.vector.max(out=cand[:, k * 8:k * 8 + 8], in_=ps[:])
                            nc.vector.max_index(out=lidx[:, k * 8:k * 8 + 8],
                                                in_max=cand[:, k * 8:k * 8 + 8],
                                                in_values=ps[:])
                        gidx_cand = smp.tile([128, NCAND], u16, tag="gcand")
                        nc.vector.tensor_tensor(out=gidx_cand[:], in0=lidx[:],
                                                in1=blockbase[:],
                                                op=mybir.AluOpType.add)

                        # exact top-32 of the 128 candidates
                        work = smp.tile([128, NCAND], f32, tag="work")
                        mxc = smp.tile([128, K], f32, tag="mxc")
                        pos = smp.tile([128, K], u16, tag="pos")
                        src = cand
                        for it in range(4):
                            nc.vector.max(out=mxc[:, it * 8:it * 8 + 8], in_=src[:])
                            nc.vector.max_index(out=pos[:, it * 8:it * 8 + 8],
                                                in_max=mxc[:, it * 8:it * 8 + 8],
                                                in_values=src[:])
                            if it < 3:
                                nc.vector.match_replace(
                                    out=work[:],
                                    in_to_replace=mxc[:, it * 8:it * 8 + 8],
                                    in_values=src[:], imm_value=NEG)
                                src = work

                        # extract global idx in rank order via two local_scatters
                        rank_at = smp.tile([128, NCAND], u16, tag="rank_at")
                        nc.gpsimd.local_scatter(
                            out_ap=rank_at[:], data_ap=ranks[:],
                            idxs_ap=pos[:].bitcast(i16),
                            channels=128, num_elems=NCAND, num_idxs=K)
                        rankm1 = smp.tile([128, NCAND], i16, tag="rankm1")
                        nc.vector.tensor_scalar(rankm1[:], rank_at[:], 1.0, None,
                                                op0=mybir.AluOpType.subtract)
                        gidx = smp.tile([128, K], u16, tag="gidx")
                        nc.gpsimd.local_scatter(
                            out_ap=gidx[:], data_ap=gidx_cand[:], idxs_ap=rankm1[:],
                            channels=128, num_elems=K, num_idxs=NCAND)

                        # ball-query mask -> replace far slots with top-1 idx
                        mask = smp.tile([128, K], u32, tag="mask")
                        nc.vector.tensor_scalar(mask[:], mxc[:], -R2, None,
                                                op0=mybir.AluOpType.is_lt)
                        nc.vector.copy_predicated(gidx[:], mask[:],
                                                  gidx[:, 0:1].to_broadcast([128, K]))

                        # stage idx lists to DRAM in ap_gather core-wrapped layout
                        nc.sync.dma_start(
                            out=idx_v[t, 0],
                            in_=gidx[0:64, :].rearrange("q (hi lo) -> q hi lo", hi=2))
                        nc.sync.dma_start(
                            out=idx_v[t, 1],
                            in_=gidx[64:128, :].rearrange("q (hi lo) -> q hi lo", hi=2))

                    # ---- gather + epilogue for the group ----
                    for t in range(g0, g0 + GRP):
                        q0 = t * 128
                        idxw = ixp.tile([128, 128], u16, tag="idxw")
                        for core in range(8):
                            nc.sync.dma_start(
                                out=idxw[core * 16:(core + 1) * 16, :],
                                in_=idx_l[t, core // 4])
                        gath = gp.tile([128, 2048], f32, tag="gath")
                        nc.gpsimd.ap_gather(
                            out_ap=gath[:].rearrange("p (s one) -> p s one", one=1),
                            in_ap=gtt[:].rearrange("p (e one) -> p e one", one=1),
                            idxs_ap=idxw[:].bitcast(i16),
                            channels=128, num_elems=N, d=1, num_idxs=2048)

                        gmax = smp.tile([128, 64], f32, tag="gmax")
                        nc.vector.reduce_max(
                            out=gmax[:],
                            in_=gath[:].rearrange("p (q s) -> p q s", s=K),
                            axis=mybir.AxisListType.X)

                        hp = hps.tile([128, 64], f32, tag="hps")
                        nc.tensor.matmul(hp[:], lhsT8[:],
                                         a8[:, t * 64:(t + 1) * 64],
                                         start=True, stop=True)
                        o = smp.tile([128, 64], f32, tag="o")
                        nc.vector.tensor_sub(o[:], gmax[:], hp[:])
                        nc.vector.tensor_scalar_max(o[:], o[:], 0.0)
                        nc.sync.dma_start(out=y_out[:, q0:q0 + 64], in_=o[0:C, :])
                        nc.sync.dma_start(out=y_out[:, q0 + 64:q0 + 128],
                                          in_=o[C:2 * C, :])

    nc.compile()
    return nc


def _get_nc():
    if "nc" not in _CACHE:
        _CACHE["nc"] = _build()
    return _CACHE["nc"]


def _make_in_maps(inputs):
    points_coor = np.ascontiguousarray(inputs["points_coor"], np.float32)
    points_fea = np.ascontiguousarray(inputs["points_fea"], np.float32)
    W = np.ascontiguousarray(inputs["W"], np.float32)
    bnt = np.ascontiguousarray(
        np.stack([inputs["gamma"], inputs["beta"], inputs["running_mean"],
                  inputs["running_var"]], axis=1), np.float32)
    blockbase = np.repeat((np.arange(NBLK, dtype=np.uint16) * 512), 8)
    blockbase = np.tile(blockbase[None, :], (128, 1)).copy()
    ranks = np.tile(np.arange(1, K + 1, dtype=np.uint16)[None, :], (128, 1)).copy()
    constrows = np.stack([np.ones(N, np.float32), -np.ones(N, np.float32)])
    in_maps = []
    for core in range(NCORES):
        b, h = core // 2, core % 2
        in_maps.append(dict(
            xyz=points_coor[b],
            xyzq=np.ascontiguousarray(points_coor[b][:, h * QPC:(h + 1) * QPC]),
            fea=points_fea[b],
            w=W,
            bnt=bnt,
            blockbase=blockbase,
            ranks=ranks,
            constrows=constrows,
        ))
    return in_maps


def kernel(points_coor, points_fea, W, gamma, beta, running_mean, running_var,
           **_unused):
    inputs = dict(points_coor=points_coor, points_fea=points_fea, W=W,
                  gamma=gamma, beta=beta, running_mean=running_mean,
                  running_var=running_var)
    nc = _get_nc()
    in_maps = _make_in_maps(inputs)
    res = run_bass_kernel_spmd(nc, in_maps, list(range(NCORES)))
    out = np.empty((B, C, N), np.float32)
    for core in range(NCORES):
        b, h = core // 2, core % 2
        out[b, :, h * QPC:(h + 1) * QPC] = res.results[core]["y"]
    return out
